# revision 1
# baseline (speedup 1.0000x reference)
"""AdaptiveRouter Trainium2 kernel — 8-core data-parallel, raw Bass.

build_kernel(nrep=N) emits N back-to-back repetitions of the full pipeline
with monotonically increasing semaphore thresholds (no sem resets needed).
nrep>1 is used for steady-state timing via wall-clock deltas; outputs stay
correct (each rep overwrites the same DRAM outputs).
"""
import sys
sys.path.insert(0, '/opt/trn_rl_repo')
import numpy as np
from contextlib import ExitStack
from concourse import bass, mybir

F32R = mybir.dt.float32r
F32 = mybir.dt.float32
U32 = mybir.dt.uint32
U8 = mybir.dt.uint8
F8 = mybir.dt.float8e4
AF = mybir.ActivationFunctionType
OP = mybir.AluOpType

T = 4096
H = 1024
E = 64
KC = 8
NJ = 8
CH = 512
NT = 4


def build_kernel(nrep=1):
    nc = bass.Bass()
    NREPNJ = nrep * NJ

    hst_e = nc.declare_dram_parameter("hst", [128, KC, T], F32R, isOutput=False)
    rwt_e = nc.declare_dram_parameter("rwt", [128, KC, E], F32R, isOutput=False)
    rwl_e = nc.declare_dram_parameter("rwl", [128, KC, E], F32R, isOutput=False)
    rw8_e = nc.declare_dram_parameter("rw8", [128, KC, E], F8, isOutput=False)
    hlo_e = nc.declare_dram_parameter("hlo", [128, KC, T], F8, isOutput=False)
    w1t_e = nc.declare_dram_parameter("w1t", [128, KC, 2, 128], F32R, isOutput=False)
    w2c_e = nc.declare_dram_parameter("w2c", [128, 2], F32, isOutput=False)
    b1c_e = nc.declare_dram_parameter("b1c", [128, 2], F32, isOutput=False)
    nb2_e = nc.declare_dram_parameter("nb2", [128, 1], F32, isOutput=False)
    idn_e = nc.declare_dram_parameter("idn", [128, 128], F32, isOutput=False)
    id12_e = nc.declare_dram_parameter("id12", [128, 128], F32, isOutput=False)
    one_e = nc.declare_dram_parameter("one", [128, 1], F32, isOutput=False)
    oh_e = nc.declare_dram_parameter("oh", [128, 4], F32, isOutput=False)

    lg_e = nc.declare_dram_parameter("logits", [128, NJ * NT, E], F32, isOutput=True)
    ws_e = nc.declare_dram_parameter("wsel", [128, NJ * NT, 8], F32, isOutput=True)
    st_e = nc.declare_dram_parameter("stats", [1, 96], F32, isOutput=True)

    es = ExitStack()
    sb = nc.sbuf_tensor
    hst = es.enter_context(sb([128, 3, KC, CH], F32R))
    rwt = es.enter_context(sb([128, KC, E], F32R))
    rwl = es.enter_context(sb([128, KC, E], F32R))
    rw8 = es.enter_context(sb([128, KC, E], F8))
    hlo = es.enter_context(sb([128, 3, KC, CH], F8))
    w1t = es.enter_context(sb([128, KC, 2, 128], F32R))
    w2c = es.enter_context(sb([128, 2], F32))
    b1c = es.enter_context(sb([128, 2], F32))
    nb2 = es.enter_context(sb([128, 1], F32))
    idn = es.enter_context(sb([128, 128], F32))
    id12 = es.enter_context(sb([128, 128], F32))
    one = es.enter_context(sb([128, 1], F32))
    oh = es.enter_context(sb([128, 4], F32))

    lT = es.enter_context(sb([64, 2, CH], F32))
    lC = es.enter_context(sb([64, 2, CH], F32))
    h1s = es.enter_context(sb([128, 2, CH], F32))
    lg = es.enter_context(sb([128, 2, NT, E], F32))
    et = es.enter_context(sb([128, 3, NT, E], F32))
    mx8 = es.enter_context(sb([128, 3, NT, 8], F32))
    mi8 = es.enter_context(sb([128, 3, NT, 8], U32))
    if4 = es.enter_context(sb([128, 3, NT, 8], F32))
    negm = es.enter_context(sb([128, 3, NT], F32))
    S_ = es.enter_context(sb([128, 3, NT], F32))
    T_ = es.enter_context(sb([128, 3, NT], F32))
    rS = es.enter_context(sb([128, 3, NT], F32))
    lgS = es.enter_context(sb([128, 3, NT], F32))
    e4 = es.enter_context(sb([128, 3, NT, 4], F32))
    S4 = es.enter_context(sb([128, 3, NT], F32))
    rS4 = es.enter_context(sb([128, 3, NT], F32))
    w4 = es.enter_context(sb([128, 3, NT, 8], F32))
    msk = es.enter_context(sb([128, 3, NT], U8))
    tmp4 = es.enter_context(sb([128, NT], F32))
    entb = es.enter_context(sb([128, NJ * NT], F32))
    wsel = es.enter_context(sb([128, 2, NT, 8], F32))
    sout = es.enter_context(sb([1, 96], F32))
    scr = es.enter_context(sb([128, E], F32))

    psR = es.enter_context(nc.psum_tensor([64, 2, CH], F32))
    psH = es.enter_context(nc.psum_tensor([128, 2, CH], F32))
    psLQ = es.enter_context(nc.psum_tensor([128, 2, CH], F32))
    psS = es.enter_context(nc.psum_tensor([1, 512], F32))
    psC = es.enter_context(nc.psum_tensor([64, 512], F32))

    sem_names = ["dmaw", "dmah0", "dmah1", "dmah2", "dmao_lg", "dmao_ws",
                 "dmao_st", "s_r", "s_w1", "s_c", "s_tr", "s_relu",
                 "s_cr", "s_cl", "s_exp", "s_e4", "s_lgS", "s_mx", "s_negm",
                 "s_stt", "s_rS", "s_msk", "s_wsel", "s_ent", "s_el",
                 "s_entmm", "s_fin", "dmal0", "dmal1", "dmal2", "s_cmb"]
    sems = {n: es.enter_context(nc.semaphore(n)) for n in sem_names}
    g = lambda n: sems[n]

    block = es.enter_context(nc.Block())

    @block.sync
    def _(sync):
        for t_, e_ in [(rwt, rwt_e), (w1t, w1t_e), (w2c, w2c_e), (b1c, b1c_e),
                       (nb2, nb2_e), (idn, idn_e), (one, one_e), (oh, oh_e),
                       (rwl, rwl_e), (rw8, rw8_e), (id12, id12_e)]:
            sync.dma_start(t_[:], e_[:]).then_inc(g("dmaw"), 16)
        for jo in range(NREPNJ + 2):
            if jo < NREPNJ:
                j = jo
                jj = j % NJ
                if j >= 3:
                    sync.wait_ge(g("s_w1"), j - 2)
                sync.dma_start(hst[:, j % 3, :, :],
                               hst_e[:, :, jj * CH:(jj + 1) * CH]
                               ).then_inc(g(f"dmah{j % 3}"), 16)
                sync.dma_start(hlo[:, j % 3, :, :],
                               hlo_e[:, :, jj * CH:(jj + 1) * CH]
                               ).then_inc(g(f"dmal{j % 3}"), 16)
            if 0 <= jo - 2 < NREPNJ:
                j = jo - 2
                jj = j % NJ
                b = j % 2
                sync.wait_ge(g("s_cl"), 4 * (j + 1))
                if j >= 1:
                    sync.wait_ge(g("dmao_lg"), 16 * j)
                sync.dma_start(lg_e[:, jj * NT:(jj + 1) * NT, :], lg[:, b, :, :]
                               ).then_inc(g("dmao_lg"), 16)
                sync.wait_ge(g("s_wsel"), j + 1)
                if j >= 1:
                    sync.wait_ge(g("dmao_ws"), 16 * j)
                sync.dma_start(ws_e[:, jj * NT:(jj + 1) * NT, :],
                               wsel[:, b, :, :]
                               ).then_inc(g("dmao_ws"), 16)
                if jj == NJ - 1:
                    r = j // NJ
                    sync.wait_ge(g("s_fin"), r + 1)
                    sync.dma_start(st_e[:], sout[:]).then_inc(g("dmao_st"), 16)
        sync.wait_ge(g("dmaw"), 176)
        for q in range(3):
            n_on_q = len([j for j in range(NREPNJ) if j % 3 == q])
            sync.wait_ge(g(f"dmah{q}"), 16 * n_on_q)
            sync.wait_ge(g(f"dmal{q}"), 16 * n_on_q)
        sync.wait_ge(g("dmao_lg"), 16 * NREPNJ)
        sync.wait_ge(g("dmao_ws"), 16 * NREPNJ)
        sync.wait_ge(g("dmao_st"), 16 * nrep)

    @block.tensor
    def _(tensor):
        def pe_iter(j):
            if j < NREPNJ:
                jb = j % 3
                b = j % 2
                if j == 0:
                    tensor.wait_ge(g("dmaw"), 176)
                tensor.wait_ge(g(f"dmah{j % 3}"), 16 * (j // 3 + 1))
                tensor.wait_ge(g(f"dmal{j % 3}"), 16 * (j // 3 + 1))
                if j >= 2:
                    tensor.wait_ge(g("s_cr"), j - 1)
                for k in range(KC):
                    tensor.matmul(psR[0:64, b, :], rwt[:, k, :],
                                  hst[:, jb, k, :],
                                  start=(k == 0), stop=False)
                for k in range(KC):
                    tensor.matmul(psR[0:64, b, :], rwl[:, k, :],
                                  hst[:, jb, k, :],
                                  start=False, stop=(k == KC - 1))
                if j >= 1:
                    tensor.wait_ge(g("s_cmb"), j)
                for k in range(KC):
                    mm = tensor.matmul(psC[0:64, :], rw8[:, k, :],
                                       hlo[:, jb, k, :],
                                       start=(k == 0), stop=(k == KC - 1),
                                       skip_group_check=True)
                mm.then_inc(g("s_r"), 1)
                if j >= 1:
                    tensor.wait_ge(g("s_relu"), j)
                for m in range(2):
                    for k in range(KC):
                        mm = tensor.matmul(psH[:, m, :], w1t[:, k, m, :],
                                           hst[:, jb, k, :],
                                           start=(k == 0), stop=(k == KC - 1))
                mm.then_inc(g("s_w1"), 1)
            if 0 <= j - 1 < NREPNJ:
                i = j - 1
                b = i % 2
                tensor.wait_ge(g("s_relu"), i + 1)
                if i >= 2:
                    tensor.wait_ge(g("s_cl"), 4 * (i - 1))
                    tensor.wait_ge(g("s_msk"), i - 1)
                for t in range(NT):
                    for m in range(2):
                        mm = tensor.matmul(psLQ[:, b, 256 + t:257 + t],
                                           h1s[:, m, t * 128:(t + 1) * 128],
                                           w2c[:, m:m + 1],
                                           start=(m == 0), stop=(m == 1),
                                           skip_group_check=True)
                mm.then_inc(g("s_c"), 1)
                tensor.wait_ge(g("s_cr"), i + 1)
                for t in range(NT):
                    tensor.matmul(psLQ[:, b, t * 64:(t + 1) * 64],
                                  lT[0:64, b, t * 128:(t + 1) * 128],
                                  idn[0:64, 0:64], is_transpose=True,
                                  start=True, stop=False,
                                  skip_group_check=True
                                  ).then_inc(g("s_tr"), 1)
                    tensor.matmul(psLQ[:, b, t * 64:(t + 1) * 64],
                                  lC[0:64, b, t * 128:(t + 1) * 128],
                                  idn[0:64, 0:64], is_transpose=True,
                                  start=False, stop=True,
                                  skip_group_check=True
                                  ).then_inc(g("s_tr"), 1)
            if 0 <= j - 2 < NREPNJ:
                i = j - 2
                b3 = i % 3
                rep = i // NJ
                tensor.wait_ge(g("s_rS"), i + 1)
                tensor.wait_ge(g("s_exp"), 4 * (i + 1))
                if i % NJ == 0 and rep >= 1:
                    tensor.wait_ge(g("s_fin"), rep)
                for t in range(NT):
                    mm = tensor.matmul(psS[0:1, 0:64], rS[:, b3, t:t + 1],
                                       et[:, b3, t, :],
                                       start=(i % NJ == 0 and t == 0),
                                       stop=(i % NJ == NJ - 1 and t == NT - 1),
                                       skip_group_check=True)
                mm.then_inc(g("s_el"), 1)
            if 0 <= j - 2 < NREPNJ and (j - 2) % NJ == NJ - 1:
                rep = (j - 2) // NJ
                tensor.wait_ge(g("s_ent"), NJ * (rep + 1))
                tensor.matmul(psS[0:1, 128:160], one[:, 0:1], entb[:, :],
                              start=True, stop=True, skip_group_check=True
                              ).then_inc(g("s_entmm"), 1)

        for j in range(NREPNJ + 2):
            pe_iter(j)

    @block.scalar
    def _(scalar):
        def act_iter(j):
            if j < NREPNJ:
                b = j % 2
                scalar.wait_ge(g("s_w1"), j + 1)
                if j >= 1:
                    scalar.wait_ge(g("s_c"), j)
                for m in range(2):
                    scalar.activation(h1s[:, m, :], psH[:, m, :], AF.Relu,
                                      bias=b1c[:, m:m + 1], scale=1.0)
                    scalar.drain()
                scalar.sem_inc(g("s_relu"), 1)
                scalar.wait_ge(g("s_r"), j + 1)
                if j >= 2:
                    scalar.wait_ge(g("s_tr"), 8 * (j - 1))
                scalar.copy(lT[0:64, b, :], psR[0:64, b, :])
                scalar.drain()
                scalar.activation(lC[0:64, b, :], psC[0:64, :], AF.Copy,
                                  scale=float(2.0 ** -12))
                scalar.drain()
                scalar.sem_inc(g("s_cr"), 1)
                scalar.sem_inc(g("s_cmb"), 1)

            if 0 <= j - 1 < NREPNJ:
                i = j - 1
                b = i % 2
                b3 = i % 3
                scalar.wait_ge(g("s_tr"), 8 * (i + 1))
                if i >= 2:
                    scalar.wait_ge(g("s_stt"), i - 1)
                    scalar.wait_ge(g("dmao_lg"), 16 * (i - 1))
                for t in range(NT):
                    scalar.copy(lg[:, b, t, :], psLQ[:, b, t * 64:(t + 1) * 64])
                    scalar.drain()
                    scalar.sem_inc(g("s_cl"), 1)
                scalar.wait_ge(g("s_negm"), i + 1)
                if i >= 3:
                    scalar.wait_ge(g("s_el"), i - 2)
                for t in range(NT):
                    scalar.activation(et[:, b3, t, :], lg[:, b, t, :], AF.Exp,
                                      bias=negm[:, b3, t:t + 1], scale=1.0,
                                      accum_out=S_[:, b3, t:t + 1])
                    scalar.drain()
                    scalar.sem_inc(g("s_exp"), 1)
                for t in range(NT):
                    scalar.wait_ge(g("s_mx"), 4 * i + t + 1)
                    scalar.activation(e4[:, b3, t, :], mx8[:, b3, t, 0:4], AF.Exp,
                                      bias=negm[:, b3, t:t + 1], scale=1.0)
                    scalar.drain()
                scalar.sem_inc(g("s_e4"), 1)
                scalar.activation(lgS[:, b3, :], S_[:, b3, :], AF.Ln)
                scalar.drain()
                scalar.sem_inc(g("s_lgS"), 1)

        for j in range(NREPNJ + 1):
            act_iter(j)

    @block.vector
    def _(vector):
        def dve_iter(i):
            b = i % 2
            b3 = i % 3
            ii = i % NJ
            rep = i // NJ
            for t in range(NT):
                vector.wait_ge(g("s_cl"), 4 * i + t + 1)
                vector.max(mx8[:, b3, t, :], lg[:, b, t, :])
                vector.drain()
                vector.max_index(mi8[:, b3, t, :], mx8[:, b3, t, :],
                                 lg[:, b, t, :])
                vector.drain()
                vector.sem_inc(g("s_mx"), 1)
            vector.tensor_scalar_mul(
                negm[:, b3, :],
                mx8[:, b3, :, 0:1].rearrange("p a one -> p (a one)"), -1.0)
            vector.drain()
            vector.sem_inc(g("s_negm"), 1)
            for t in range(NT):
                vector.wait_ge(g("s_exp"), 4 * i + t + 1)
                vector.scalar_tensor_tensor(scr[:], lg[:, b, t, :],
                                            negm[:, b3, t:t + 1],
                                            et[:, b3, t, :],
                                            OP.add, OP.mult,
                                            accum_out=T_[:, b3, t:t + 1])
                vector.drain()
            vector.sem_inc(g("s_stt"), 1)
            vector.reciprocal(rS[:, b3, :], S_[:, b3, :])
            vector.drain()
            vector.sem_inc(g("s_rS"), 1)
            vector.wait_ge(g("s_c"), i + 1)
            vector.wait_ge(g("s_tr"), 8 * (i + 1))
            vector.tensor_scalar(msk[:, b3, :], psLQ[:, b, 256:260],
                                 nb2[:, 0:1], None, OP.is_gt)
            vector.drain()
            vector.sem_inc(g("s_msk"), 1)
            vector.wait_ge(g("s_e4"), i + 1)
            vector.tensor_reduce(S4[:, b3, :], e4[:, b3, :, :], op=OP.add,
                                 axis=mybir.AxisListType.X)
            vector.drain()
            vector.reciprocal(rS4[:, b3, :], S4[:, b3, :])
            vector.drain()
            vector.tensor_tensor(
                w4[:, b3, :, 0:4], e4[:, b3, :, :],
                rS4[:, b3, :].rearrange("p (a one) -> p a one", one=1
                                        ).to_broadcast([128, NT, 4]), OP.mult)
            vector.drain()
            if i >= 2:
                vector.wait_ge(g("dmao_ws"), 16 * (i - 1))
            mb = msk[:, b3, :].rearrange("p (a one) -> p a one", one=1
                                         ).to_broadcast([128, NT, 4])
            vector.tensor_copy(wsel[:, b, :, 0:4], oh[:].rearrange(
                "p (one b) -> p one b", one=1).to_broadcast([128, NT, 4]))
            vector.drain()
            vector.copy_predicated(wsel[:, b, :, 0:4], mb, w4[:, b3, :, 0:4])
            vector.drain()
            vector.tensor_copy(if4[:, b3, :, 0:4], mi8[:, b3, :, 0:4])
            vector.drain()
            vector.tensor_scalar_mul(wsel[:, b, :, 4:8], if4[:, b3, :, 0:4], 0.0)
            vector.drain()
            vector.tensor_copy(wsel[:, b, :, 4:5], if4[:, b3, :, 0:1])
            vector.drain()
            vector.copy_predicated(wsel[:, b, :, 4:8], mb, if4[:, b3, :, 0:4])
            vector.drain()
            vector.sem_inc(g("s_wsel"), 1)
            vector.tensor_tensor(tmp4[:], T_[:, b3, :], rS[:, b3, :], OP.mult)
            vector.drain()
            vector.wait_ge(g("s_lgS"), i + 1)
            if ii == 0 and rep >= 1:
                vector.wait_ge(g("s_entmm"), rep)
            vector.tensor_tensor(entb[:, ii * NT:(ii + 1) * NT], lgS[:, b3, :],
                                 tmp4[:], OP.subtract)
            vector.drain()
            vector.sem_inc(g("s_ent"), 1)
            if ii == NJ - 1:
                vector.wait_ge(g("s_entmm"), rep + 1)
                if rep >= 1:
                    vector.wait_ge(g("dmao_st"), 16 * rep)
                vector.tensor_copy(sout[0:1, 0:64], psS[0:1, 0:64])
                vector.drain()
                vector.tensor_copy(sout[0:1, 64:96], psS[0:1, 128:160])
                vector.drain()
                vector.sem_inc(g("s_fin"), 1)

        for i in range(NREPNJ):
            dve_iter(i)

    es.close()
    return nc


def round_fp32r(x):
    u = np.ascontiguousarray(x, dtype=np.float32).view(np.uint32)
    lsb = (u >> 12) & np.uint32(1)
    r = (u + np.uint32(0x7FF) + lsb) & np.uint32(0xFFFFF000)
    return r.view(np.float32)


def make_in_maps(hidden_states, router_w, w1, b1, w2, b2, n_cores=8):
    import ml_dtypes
    hs32 = np.ascontiguousarray(hidden_states, dtype=np.float32)
    rw32 = np.ascontiguousarray(router_w, dtype=np.float32)
    hs_r = round_fp32r(hs32)
    rw_r = round_fp32r(rw32)
    w1_r = round_fp32r(w1)
    rw_lo = rw32 - rw_r                      # exactly fp32r-representable
    hs_lo = (hs32 - hs_r) * 4096.0           # scaled residual -> fp8 e4m3
    hs_lo8 = hs_lo.astype(ml_dtypes.float8_e4m3)
    rw_8 = rw32.astype(ml_dtypes.float8_e4m3)

    rwt = np.ascontiguousarray(rw_r.T.reshape(KC, 128, E).transpose(1, 0, 2))
    rwl = np.ascontiguousarray(rw_lo.T.reshape(KC, 128, E).transpose(1, 0, 2))
    rw8 = np.ascontiguousarray(rw_8.T.reshape(KC, 128, E).transpose(1, 0, 2))
    w1t = np.ascontiguousarray(w1_r.T.reshape(KC, 128, 2, 128).transpose(1, 0, 2, 3))
    w2c = np.ascontiguousarray(np.asarray(w2, np.float32).reshape(2, 128).T)
    b1c = np.ascontiguousarray(np.asarray(b1, np.float32).reshape(2, 128).T)
    nb2 = np.full((128, 1), -float(np.asarray(b2).reshape(-1)[0]), np.float32)
    idn = np.eye(128, dtype=np.float32)
    id12 = (np.eye(128) * 2.0 ** -12).astype(np.float32)
    one = np.ones((128, 1), np.float32)
    oh = np.zeros((128, 4), np.float32)
    oh[:, 0] = 1.0

    in_maps = []
    for c in range(n_cores):
        shard = hs_r[c * T:(c + 1) * T]
        hst = np.ascontiguousarray(shard.T.reshape(KC, 128, T).transpose(1, 0, 2))
        shard8 = hs_lo8[c * T:(c + 1) * T]
        hlo = np.ascontiguousarray(shard8.T.reshape(KC, 128, T).transpose(1, 0, 2))
        in_maps.append({"hst": hst, "hlo": hlo, "rwt": rwt, "rwl": rwl,
                        "rw8": rw8, "w1t": w1t, "w2c": w2c,
                        "b1c": b1c, "nb2": nb2, "idn": idn, "id12": id12,
                        "one": one, "oh": oh})
    return in_maps


def assemble_outputs(results, n_cores=8):
    N = n_cores * T
    logits = np.empty((N, E), np.float32)
    wts = np.empty((N, 4), np.float32)
    sel = np.empty((N, 4), np.int32)
    el_sum = np.zeros(E, np.float64)
    ent_sum = 0.0
    for c, r in enumerate(results):
        lgv = r["logits"].reshape(128, NJ, NT, E).transpose(1, 2, 0, 3
                                                           ).reshape(T, E)
        wsv = r["wsel"].reshape(128, NJ, NT, 8).transpose(1, 2, 0, 3
                                                         ).reshape(T, 8)
        logits[c * T:(c + 1) * T] = lgv
        wts[c * T:(c + 1) * T] = wsv[:, 0:4]
        sel[c * T:(c + 1) * T] = wsv[:, 4:8].astype(np.int32)
        el_sum += r["stats"][0, 0:64].astype(np.float64)
        ent_sum += float(r["stats"][0, 64:96].sum())
    expert_load = el_sum / N
    load_variance = np.float32(expert_load.var(ddof=1))
    entropy = np.float32(ent_sum / N)
    return logits, sel, wts, load_variance, entropy
